# revision 19
# baseline (speedup 1.0000x reference)
"""Causal self-attention Trainium2 Bass kernel (v3, bf16 pipeline).

Problem: nn_CausalSelfAttention (B=2, L=2048, D=1024, H=16 heads, Khd=64).

Sharding (8 cores): data-parallel over B (2 way) x tensor-parallel over
heads (4 way, 4 heads/core).  Each core computes
  qkv_local = x_b @ W_attn_local.T          (c_attn column-sharded)
  attn_local = causal_attention(q,k,v)      (4 heads)
  y_partial  = attn_local @ W_proj_local.T  (c_proj row-sharded)
and the host sums the 4 partials per batch (the row-parallel unshard).

v3 changes over v2 (~190us -> ~130us HW):
  - softmax normalization without gpsimd partition_broadcast: v_aug
    carries 64 REPLICATED ones-columns (cols 0:64; v in 64:128), so the
    PV matmul materializes the denominator on outT partitions 0:64 and
    normalize is reciprocal_approx_fast + tensor_mul on DVE only.
    HW-measured: gpsimd broadcast [64,512] ~4.8us (10x the model) and
    the accurate DVE reciprocal is an iterative 8-cyc/elem op - the old
    chain was the kernel's hidden serializer.  M=128 PV streams at the
    same rate as M=65 (bf16 matmul: 2 cols/cycle, 134ns at N=512).
  - custom-DVE ops (the fast reciprocal) only work at base partition 0
    on HW (cross-base reads garbage; base-64 wedges the device), hence
    ones-first v_aug layout.
  - fp8/DoubleRow was evaluated and REJECTED: any e4m3 quantization on
    the value path costs ~3e-2 absmax-rel error alone (quantization
    noise in random-sign dot products scales with the signal).
  - QKV q/k parts emitted as half-units (8 MMs, ~1.1us) used with v and
    proj units as PE filler inside attention rounds; only v0-3 + two
    qk halves precede the first scores, so the ACT exp stream (the
    ~70us serial resource) starts as early as the DMAs allow.
  - adaptive filler draining: surplus fillers are consumed 2-per-round
    so none are left at a pair boundary where they would queue ahead
    of the next pair's score matmuls and stall exp.
  - y DMAs rotate over sync/gpsimd queues.

Kept from v2: bf16 pipeline, k-interleaved host packing with 192-256KB
DMA instructions spread across queues, score/exp/PV software pipeline
with per-pair PSUM tile rotation, endgame per-chunk normalize + proj(3)
interleave.
"""

import math

import numpy as np

B, L, D, H = 2, 2048, 1024, 16
KHD = D // H  # 64 head dim
NCORES = 8
HPC = 4  # heads per core
FQK = 2 * HPC * KHD  # 512 q+k local features
FV = HPC * KHD  # 256 v local features
FQKV = FQK + FV  # 768
DK = D // 128  # 8 contraction chunks
LC = L // 128  # 16 row chunks
NJ = L // 512  # 4 qrow blocks
NXB = 4  # xT column blocks of 512
SCALE = 1.0 / math.sqrt(KHD)

_CACHE = {}

# Emission schedule: which filler units run inside which attention pair
# (one unit consumed per round).  v8..v15 are QKV value-chunk units;
# p<j>.<lq><half> are proj halves; None skips a round.
SCHED = {
    "plan": {
        "00": ["q0.01", "q2.01", "q1.00", "q3.00"],
        "02": ["q1.01", "q3.01", "v4"],
        "10": ["v5", "v6", "v7", "v8", "v9"],
        "12": ["v10", "v11", "q0.10", "q2.10", "q0.11"],
        "20": ["q2.11", "q1.10", "q3.10", "v12", "v13", "p0.00", "p0.01"],
        "22": ["q1.11", "q3.11", "v14", "v15", "p0.10", "p0.11", "p0.20"],
        "30": ["p0.21", "p0.30", "p0.31", "p1.00", "p1.01", "p1.10", "p1.11",
               "p1.20", "p1.21"],
        "32": ["p1.30", "p1.31", "p2.00", "p2.01", "p2.10", "p2.11", "p2.20",
               "p2.21", "p2.30", "p2.31"],
    },
    "tail": [],  # proj(3) is emitted inside the endgame pair
}


def _build(has_bqk: bool, has_bv: bool, has_bp: bool, reps: int = 1):
    import concourse.bass as bass
    import concourse.mybir as mybir
    import concourse.tile as tile
    from concourse import bacc

    f32 = mybir.dt.float32
    bf16 = mybir.dt.bfloat16

    nc = bacc.Bacc(None, target_bir_lowering=False)
    # host-packed k-interleaved layouts: one wide DMA per tensor
    xTi_d = nc.declare_dram_parameter("xTi", [128, NXB, DK, 512], bf16, isOutput=False)
    wqkvTi_d = nc.declare_dram_parameter("wqkvTi", [128, DK, FQKV], bf16, isOutput=False)
    wpTi_d = nc.declare_dram_parameter("wpTi", [128, 2, D], bf16, isOutput=False)
    tri_d = nc.declare_dram_parameter("tri", [128, 128], bf16, isOutput=False)
    if has_bqk:
        bqk_d = nc.declare_dram_parameter("bqk", [128, FQK // 128], f32, isOutput=False)
    if has_bv or has_bp:
        onesrow_d = nc.declare_dram_parameter("onesrow", [1, 128], bf16, isOutput=False)
    if has_bv:
        bv_d = nc.declare_dram_parameter("bv", [1, FV], bf16, isOutput=False)
    if has_bp:
        bp_d = nc.declare_dram_parameter("bp", [1, D], bf16, isOutput=False)
    y_d = nc.declare_dram_parameter("y", [L, D], bf16, isOutput=True)

    with nc.allow_low_precision(reason="bf16 matmul pipeline, fp32 accum"), tile.TileContext(nc) as tc:
        with (
            tc.tile_pool(name="persist", bufs=1) as persist,
            tc.tile_pool(name="work", bufs=3) as work,
            tc.tile_pool(name="small", bufs=2) as small,
            tc.tile_pool(name="ps_sc", bufs=2, space="PSUM") as ps_sc,
            tc.tile_pool(name="ps_out", bufs=2, space="PSUM") as ps_out,
            tc.tile_pool(name="ps_y", bufs=2, space="PSUM") as ps_y,
        ):
            for _rep in range(reps):
                # ---- persistent SBUF tensors ----
                xTi_sbs = [
                    persist.tile([128, DK, 512], bf16, name=f"xTi{c}", tag=f"xTi{c}")
                    for c in range(NXB)
                ]
                wqkvTi_sb = persist.tile([128, DK, FQKV], bf16, tag="wqkvTi")
                qT_p = [persist.tile([128, L], bf16, name=f"qT{p}", tag=f"qT{p}") for p in range(2)]
                kT_p = [persist.tile([128, L], bf16, name=f"kT{p}", tag=f"kT{p}") for p in range(2)]
                # v augmented with 64 ones-columns (cols 0:64; v in cols
                # 64:128): the PV matmul then materializes the softmax
                # denominator replicated on outT partitions 0:64, so
                # normalization needs no partition broadcast (gpsimd
                # broadcast measured ~4.8us on HW - 10x the modeled cost -
                # and serialized every pair).  The denominator lives at
                # base partition 0 because custom-DVE ops (the fast
                # reciprocal) are broken/wedge the device at base 64.
                # M=128 streams at the same rate as M=65 (measured 134ns
                # for M=128 N=512 bf16).
                v_augs = [
                    persist.tile([128, HPC, 128], bf16, name=f"vaug{lc}", tag=f"vaug{lc}")
                    for lc in range(LC)
                ]
                attnT_js = [
                    persist.tile([128, 2, 512], bf16, name=f"attnT{j}", tag=f"attnT{j}")
                    for j in range(NJ)
                ]
                wpTi_sb = persist.tile([128, 2, D], bf16, tag="wpTi")
                tri_sb = persist.tile([128, 128], bf16)

                # ---- input DMAs: few, wide, priority-ordered; none on ACT
                # (it needs every cycle for exp).  First emit_v(0..3) needs
                # only xTi[0] cols 0:128-per-k and wqkvTi v-cols: load those
                # first so PE starts ~3us in.
                # ~192-256KB per DMA instruction: each instruction is
                # served by ~one DMA engine (~22-27 GB/s), so aggregate
                # bandwidth comes from many concurrent instructions spread
                # over the sync/scalar HWDGE + gpsimd SWDGE queues.  All
                # slices keep >=1.5KB contiguous rows (128-256 descriptors).
                engs = [nc.sync, nc.scalar, nc.gpsimd]
                # tiny k=0 pieces first: the very first emit_v matmul needs
                # only wqkv v-cols of k=0 and xTi[0] k=0, so PE starts ~2us
                # earlier than waiting for full 192-256KB chunks
                nc.sync.dma_start(
                    out=wqkvTi_sb[:, 0, FQK:FQKV], in_=wqkvTi_d[:, 0, FQK:FQKV]
                )
                nc.scalar.dma_start(
                    out=xTi_sbs[0][:, 0], in_=xTi_d[:, 0, 0]
                )
                nc.gpsimd.dma_start(
                    out=wqkvTi_sb[:, 0, 0:FQK], in_=wqkvTi_d[:, 0, 0:FQK]
                )
                for k in range(1, DK):
                    engs[k % 3].dma_start(
                        out=wqkvTi_sb[:, k], in_=wqkvTi_d[:, k]
                    )
                nc.sync.dma_start(
                    out=xTi_sbs[0][:, 1], in_=xTi_d[:, 0, 1]
                )
                for c in range(NXB):
                    for kp in range(1 if c == 0 else 0, 4):
                        engs[(c + kp) % 3].dma_start(
                            out=xTi_sbs[c][:, 2 * kp:2 * kp + 2],
                            in_=xTi_d[:, c, 2 * kp:2 * kp + 2],
                        )
                for kc in range(2):
                    engs[kc].dma_start(
                        out=wpTi_sb[:, kc], in_=wpTi_d[:, kc]
                    )
                nc.sync.dma_start(out=tri_sb, in_=tri_d[:])
                # ones columns of v_aug (softmax denominator trick,
                # replicated so the denominator lands on 64 partitions)
                for lc in range(LC):
                    nc.vector.memset(v_augs[lc][:, :, 0:KHD], 1.0)
                # PE p-state warm-up: ~2.5us of throwaway matmuls on memset
                # data so the 3us continuous-busy ramp to 2.4GHz completes
                # before real work arrives (first real matmul waits ~3us of
                # DMA anyway; the ramp would otherwise restart on it)
                if _rep == 0:
                    warm_sb = work.tile([128, 128], bf16, tag="warm", bufs=1)
                    nc.vector.memset(warm_sb, 0.0)
                    warm_ps = ps_sc.tile([128, 128], f32, tag="sc", name="warm")
                    for _ in range(27):
                        nc.tensor.matmul(
                            warm_ps, warm_sb, warm_sb, start=True, stop=True
                        )
                if has_bqk:
                    bqk_sb = persist.tile([128, FQK // 128], f32)
                    nc.sync.dma_start(out=bqk_sb, in_=bqk_d[:])
                if has_bv or has_bp:
                    ones_row = persist.tile([1, 128], bf16)
                    nc.sync.dma_start(out=ones_row, in_=onesrow_d[:])
                if has_bv:
                    bv_sb = persist.tile([1, FV], bf16)
                    nc.sync.dma_start(out=bv_sb, in_=bv_d[:])
                if has_bp:
                    bp_sb = persist.tile([1, D], bf16)
                    nc.sync.dma_start(out=bp_sb, in_=bp_d[:])

                # ---- emission helpers ----
                def emit_qk_half(m, n2, half):
                    # q,k feature-major: chunk m covers feats 128m..128m+127;
                    # (n2, half) covers L-columns 1024*n2+512*half..+512.
                    # Half-sized units (~1.1us PE) so a filler never delays
                    # the next round's score matmuls (which gate the ACT
                    # exp stream) by more than ~1us.
                    dst = qT_p if m < 2 else kT_p
                    plane = m % 2
                    ps = ps_sc.tile([128, 1024], f32, tag="sc", name="ps")
                    n = 2 * n2 + half
                    for k in range(DK):
                        nc.tensor.matmul(
                            ps[:, 0:512],
                            wqkvTi_sb[:, k, m * 128:(m + 1) * 128],
                            xTi_sbs[n][:, k, :],
                            start=(k == 0),
                            stop=(k == DK - 1),
                        )
                    csl = slice(n2 * 1024 + half * 512, n2 * 1024 + half * 512 + 512)
                    if has_bqk:
                        nc.scalar.activation(
                            dst[plane][:, csl],
                            ps[:, 0:512],
                            mybir.ActivationFunctionType.Copy,
                            bias=bqk_sb[:, m:m + 1],
                        )
                    else:
                        nc.vector.tensor_copy(
                            out=dst[plane][:, csl], in_=ps[:, 0:512]
                        )

                def emit_v(lc):
                    # v natural layout [L, feat]; uses the proj pool's PSUM
                    # slots (idle during QKV and early attention) so v units
                    # never perturb the score/exp tile rotation
                    psv = ps_y.tile([128, 512], f32, tag="psy", name="psv")
                    c, r = divmod(lc, NXB)
                    for k in range(DK):
                        nc.tensor.matmul(
                            psv[:, 0:FV],
                            xTi_sbs[c][:, k, r * 128:(r + 1) * 128],
                            wqkvTi_sb[:, k, FQK:FQKV],
                            start=(k == 0),
                            stop=(k == DK - 1) and not has_bv,
                        )
                    if has_bv:
                        nc.tensor.matmul(
                            psv[:, 0:FV], ones_row[0:1, :], bv_sb,
                            start=False, stop=True,
                        )
                    nc.vector.tensor_copy(
                        out=v_augs[lc][:, :, KHD:128],
                        in_=psv[:, 0:FV].rearrange("p (h k) -> p h k", h=HPC),
                    )

                def emit_att_pair(j, h0, fillers=(), endgame=False):
                    # two heads (same q/k plane) processed in lockstep, with the
                    # PV matmuls software-pipelined one round behind the scores
                    # so PE never waits on ScalarE's exp.  One filler thunk
                    # (a proj half) is emitted after each round as PE slack.
                    # endgame (last pair): normalize per 128-col chunk as soon
                    # as that chunk's PV accumulation is complete, and emit
                    # proj(NJ-1) column-chunks right behind - shortens the
                    # critical tail chain.
                    fillers = list(fillers)
                    pl = h0 // 2
                    pos = [(h0 % 2) * 64, ((h0 + 1) % 2) * 64]
                    heads = [h0, h0 + 1]
                    outTs = [
                        ps_out.tile([128, 512], f32, tag="outT", name="outT")
                        for _ in range(2)
                    ]
                    qrs = slice(j * 512, (j + 1) * 512)
                    last_c = 4 * j + 3
                    # rounds: below-diagonal pairs then two packed diagonal pairs
                    rounds = [("below", cp) for cp in range(0, 4 * j, 2)]
                    rounds += [("diag", 0), ("diag", 2)]
                    pending = []  # (hh, [(c, colslice in ex)]), ex tile

                    def flush_pending():
                        # endgame: close the accumulation group at c=4j+1 so
                        # cols 0:256 are readable before diag-2 PV lands
                        # (stop is sim bookkeeping only; hardware PSUM
                        # accumulation is per-address)
                        for hh, parts, ex in pending:
                            for c, exsl, n0 in parts:
                                nc.tensor.matmul(
                                    outTs[hh][:, n0:512],
                                    v_augs[c][:, heads[hh], :],
                                    ex[:, exsl],
                                    start=(c == 0),
                                    stop=(c == last_c or (endgame and c == last_c - 2)),
                                    skip_group_check=(endgame and c > last_c - 2),
                                )
                        pending.clear()

                    def emit_norm(s, w):
                        # normalize cols s*w..(s+1)*w of both heads; the
                        # denominator is already replicated on outT rows
                        # 64:128, so this is recip + mul on DVE only
                        csl = slice(s * w, (s + 1) * w)
                        for hh in range(2):
                            po = pos[hh]
                            recip = small.tile([64, w], f32, tag="recip", name="recip")
                            # approx_fast: 18 correct bits (plenty for bf16
                            # attnT), single DVE op, ~5x faster than the
                            # iterative-divide reciprocal (8 cyc/elem); all
                            # APs at base partition 0 (custom-DVE ops give
                            # garbage / wedge the device at base 64)
                            nc.vector.reciprocal_approx_fast(
                                recip, outTs[hh][0:KHD, csl]
                            )
                            nc.vector.tensor_mul(
                                attnT_js[j][po:po + 64, pl, csl],
                                outTs[hh][KHD:128, csl],
                                recip,
                            )

                    for ridx, (kind, arg) in enumerate(rounds):
                        new_pending = []
                        for hh in range(2):
                            po = pos[hh]
                            sc = ps_sc.tile([128, 1024], f32, tag="sc", name="sc")
                            if kind == "below":
                                cp = arg
                                for half in range(2):
                                    c = cp + half
                                    nc.tensor.matmul(
                                        sc[:, half * 512:(half + 1) * 512],
                                        kT_p[pl][po:po + 64, c * 128:(c + 1) * 128],
                                        qT_p[pl][po:po + 64, qrs],
                                        start=True,
                                        stop=True,
                                    )
                                ex = work.tile([128, 1024], bf16, tag="expT", name="ex", bufs=7)
                                nc.scalar.activation(
                                    ex, sc,
                                    mybir.ActivationFunctionType.Exp, scale=SCALE,
                                )
                                parts = [
                                    (cp, slice(0, 512), 0),
                                    (cp + 1, slice(512, 1024), 0),
                                ]
                            else:
                                i0 = arg
                                ws = [512 - 128 * (i0 + di) for di in range(2)]
                                offs = [0, ws[0]]
                                wtot = ws[0] + ws[1]
                                for di in range(2):
                                    c = 4 * j + i0 + di
                                    n0 = 128 * (i0 + di)
                                    nc.tensor.matmul(
                                        sc[:, offs[di]:offs[di] + ws[di]],
                                        kT_p[pl][po:po + 64, c * 128:(c + 1) * 128],
                                        qT_p[pl][po:po + 64, j * 512 + n0:(j + 1) * 512],
                                        start=True,
                                        stop=True,
                                    )
                                ex = work.tile([128, 1024], bf16, tag="expT", name="ex", bufs=7)
                                nc.scalar.activation(
                                    ex[:, 0:wtot], sc[:, 0:wtot],
                                    mybir.ActivationFunctionType.Exp, scale=SCALE,
                                )
                                for di in range(2):
                                    nc.vector.tensor_mul(
                                        ex[:, offs[di]:offs[di] + 128],
                                        ex[:, offs[di]:offs[di] + 128],
                                        tri_sb,
                                    )
                                parts = [
                                    (4 * j + i0, slice(0, ws[0]), 128 * i0),
                                    (
                                        4 * j + i0 + 1,
                                        slice(offs[1], offs[1] + ws[1]),
                                        128 * (i0 + 1),
                                    ),
                                ]
                            new_pending.append((hh, parts, ex))
                        flush_pending()
                        pending.extend(new_pending)
                        # first round has no PV yet (pipelined one behind):
                        # absorb the exp wait with an extra filler; also
                        # drain surplus fillers early so none are left at
                        # the pair boundary (a leftover would sit in the PE
                        # queue ahead of the next pair's score matmuls and
                        # stall the ACT exp stream)
                        remaining = len(rounds) - ridx - 1
                        take = 2 if (ridx == 0 or len(fillers) > remaining) else 1
                        for _ in range(take):
                            if fillers:
                                fu = fillers.pop(0)
                                if fu is not None:
                                    fu()
                        if endgame and ridx == len(rounds) - 1:
                            # diag-0 PV just flushed: cols 0:256 are final
                            for s in range(2):
                                emit_norm(s, 128)
                            for lq in range(2):
                                for h in range(2):
                                    emit_proj_half(NJ - 1, lq, h)
                    flush_pending()
                    for f in fillers:
                        if f is not None:
                            f()
                    # normalize: attnT[f, qrow] = outT[f, qrow] / outT[64, qrow]
                    if endgame:
                        for s in range(2, 4):
                            emit_norm(s, 128)
                        for lq in range(2, 4):
                            for h in range(2):
                                emit_proj_half(NJ - 1, lq, h)
                    elif j == NJ - 1:
                        for s in range(4):
                            emit_norm(s, 128)
                    else:
                        emit_norm(0, 512)

                proj_ysb = {}

                def emit_proj_half(j, lq, half):
                    # one 512-col half of one 128-row chunk of proj(j);
                    # small unit usable as PE filler between att rounds
                    lc = 4 * j + lq
                    if half == 0:
                        proj_ysb[j] = work.tile(
                            [128, 1024], bf16, tag="ysb", name="ysb", bufs=3
                        )
                    ysb = proj_ysb[j]
                    sl = slice(half * 512, (half + 1) * 512)
                    psy = ps_y.tile([128, 512], f32, tag="psy", name="psy")
                    for kc in range(2):
                        nc.tensor.matmul(
                            psy,
                            attnT_js[j][:, kc, lq * 128:(lq + 1) * 128],
                            wpTi_sb[:, kc, sl],
                            start=(kc == 0),
                            stop=(kc == 1) and not has_bp,
                        )
                    if has_bp:
                        nc.tensor.matmul(
                            psy, ones_row[0:1, :], bp_sb[0:1, sl],
                            start=False, stop=True,
                        )
                    if j == NJ - 1 and (lq * 2 + half) % 2 == 1:
                        # tail: ACT is free once exp is done; alternate
                        # copies DVE/ACT so proj(3) is PE-paced
                        nc.scalar.activation(
                            ysb[:, sl], psy, mybir.ActivationFunctionType.Copy
                        )
                    else:
                        nc.vector.tensor_copy(out=ysb[:, sl], in_=psy)
                    # only SP/ACT/gpsimd queues can start DMAs; keep ACT
                    # free for exp - rotate sync/gpsimd
                    yengs = [nc.sync, nc.gpsimd, nc.sync, nc.gpsimd]
                    if j == NJ - 1 and lq == 3:
                        # drain the final block's halves eagerly
                        yengs[half].dma_start(
                            out=y_d[lc * 128:(lc + 1) * 128, sl], in_=ysb[:, sl]
                        )
                    elif half == 1:
                        yengs[lc % 4].dma_start(
                            out=y_d[lc * 128:(lc + 1) * 128, :], in_=ysb
                        )

                def proj_fillers(j):
                    return [
                        (lambda j=j, lq=lq, h=h: emit_proj_half(j, lq, h))
                        for lq in range(4)
                        for h in range(2)
                    ]

                # ---- emission order: emit_v first so PE ramps while xT
                # streams in; attention's exp stream starts early so ACT
                # overlaps the QKV phase; proj halves injected as PE filler
                # between att rounds so the tail is only proj(3) ----
                def vu(lc):
                    return lambda: emit_v(lc)

                def qku(m, n2, half):
                    return lambda: emit_qk_half(m, n2, half)

                # SCHED["plan"]: per att pair, list of filler unit names;
                # remaining proj units run after their position's pair.
                units = {}
                for lc in range(4, LC):
                    units[f"v{lc}"] = vu(lc)
                for m in range(4):
                    for n2 in range(2):
                        for h in range(2):
                            units[f"q{m}.{n2}{h}"] = qku(m, n2, h)
                for pj in range(NJ):
                    for lq in range(4):
                        for h in range(2):
                            units[f"p{pj}.{lq}{h}"] = (
                                lambda j=pj, lq=lq, h=h: emit_proj_half(j, lq, h)
                            )
                plan = SCHED["plan"]

                def F(pair):
                    return [units[nm] if nm else None for nm in plan.get(pair, [])]

                for lc in range(4):
                    emit_v(lc)
                emit_qk_half(0, 0, 0)
                emit_qk_half(2, 0, 0)
                emit_att_pair(0, 0, fillers=F("00"))
                emit_att_pair(0, 2, fillers=F("02"))
                emit_att_pair(1, 0, fillers=F("10"))
                emit_att_pair(1, 2, fillers=F("12"))
                emit_att_pair(2, 0, fillers=F("20"))
                emit_att_pair(2, 2, fillers=F("22"))
                emit_att_pair(3, 0, fillers=F("30"))
                emit_att_pair(3, 2, fillers=F("32"), endgame=True)
                for nm in SCHED["tail"]:
                    units[nm]()

    nc.compile()
    return nc


def kernel(input_BLD, W_attn, b_attn, W_proj, b_proj):
    from ml_dtypes import bfloat16

    input_BLD = np.asarray(input_BLD, dtype=np.float32)
    W_attn = np.asarray(W_attn, dtype=np.float32)
    b_attn = np.asarray(b_attn, dtype=np.float32)
    W_proj = np.asarray(W_proj, dtype=np.float32)
    b_proj = np.asarray(b_proj, dtype=np.float32)

    has_bqk = bool(np.any(b_attn[: 2 * D]))
    has_bv = bool(np.any(b_attn[2 * D:]))
    has_bp = bool(np.any(b_proj))

    key = (has_bqk, has_bv, has_bp)
    if key not in _CACHE:
        _CACHE[key] = _build(*key)
    nc = _CACHE[key]

    tri = (np.arange(128)[None, :] >= np.arange(128)[:, None]).astype(bfloat16)
    in_maps = []
    for c in range(NCORES):
        b, t = divmod(c, 4)
        hs = t * HPC * KHD  # feature offset of this core's heads
        w_loc = np.concatenate(
            [
                W_attn[hs:hs + FV],  # q rows
                W_attn[D + hs:D + hs + FV],  # k rows
                W_attn[2 * D + hs:2 * D + hs + FV],  # v rows
            ],
            axis=0,
        )  # [768, 1024]
        # k-interleaved host packing: xTi[p, c, k, l] = x[b][c*512+l, k*128+p]
        xT = np.ascontiguousarray(input_BLD[b].T.astype(bfloat16))  # [D, L]
        xTi = np.ascontiguousarray(
            xT.reshape(DK, 128, NXB, 512).transpose(1, 2, 0, 3)
        )  # [128, 4, 8, 512]
        wqkvT = w_loc.T.astype(bfloat16)  # [D, 768]
        wqkvTi = np.ascontiguousarray(
            wqkvT.reshape(DK, 128, FQKV).transpose(1, 0, 2)
        )  # [128, 8, 768]
        wpT = W_proj[:, hs:hs + FV].T.astype(bfloat16)  # [256, 1024]
        wpTi = np.ascontiguousarray(
            wpT.reshape(2, 128, D).transpose(1, 0, 2)
        )  # [128, 2, 1024]
        m = {
            "xTi": xTi,
            "wqkvTi": wqkvTi,
            "wpTi": wpTi,
            "tri": tri,
        }
        if has_bqk:
            bqk = np.concatenate([b_attn[hs:hs + FV], b_attn[D + hs:D + hs + FV]])
            m["bqk"] = np.ascontiguousarray(bqk.reshape(FQK // 128, 128).T)
        if has_bv or has_bp:
            m["onesrow"] = np.ones((1, 128), bfloat16)
        if has_bv:
            m["bv"] = b_attn[2 * D + hs:2 * D + hs + FV][None, :].astype(bfloat16)
        if has_bp:
            m["bp"] = (b_proj / 4.0)[None, :].astype(bfloat16)
        in_maps.append(m)

    from concourse.bass_utils import run_bass_kernel_spmd

    globals()["_last_in_maps"] = in_maps
    res = run_bass_kernel_spmd(nc, in_maps, list(range(NCORES)))
    globals()["_LAST_RESULTS"] = res
    out = np.empty((B, L, D), dtype=np.float32)
    for b in range(B):
        acc = res.results[4 * b]["y"].astype(np.float32)
        for t in range(1, 4):
            acc = acc + res.results[4 * b + t]["y"].astype(np.float32)
        out[b] = acc
    return out



# revision 22
# speedup vs baseline: 1.0475x; 1.0475x over previous
"""Causal self-attention Trainium2 Bass kernel (v3, bf16 pipeline).

Problem: nn_CausalSelfAttention (B=2, L=2048, D=1024, H=16 heads, Khd=64).

Sharding (8 cores): data-parallel over B (2 way) x tensor-parallel over
heads (4 way, 4 heads/core).  Each core computes
  qkv_local = x_b @ W_attn_local.T          (c_attn column-sharded)
  attn_local = causal_attention(q,k,v)      (4 heads)
  y_partial  = attn_local @ W_proj_local.T  (c_proj row-sharded)
and the host sums the 4 partials per batch (the row-parallel unshard).

v3 changes over v2 (~190us -> ~130us HW):
  - softmax normalization without gpsimd partition_broadcast: v_aug
    carries 64 REPLICATED ones-columns (cols 0:64; v in 64:128), so the
    PV matmul materializes the denominator on outT partitions 0:64 and
    normalize is reciprocal_approx_fast + tensor_mul on DVE only.
    HW-measured: gpsimd broadcast [64,512] ~4.8us (10x the model) and
    the accurate DVE reciprocal is an iterative 8-cyc/elem op - the old
    chain was the kernel's hidden serializer.  M=128 PV streams at the
    same rate as M=65 (bf16 matmul: 2 cols/cycle, 134ns at N=512).
  - custom-DVE ops (the fast reciprocal) only work at base partition 0
    on HW (cross-base reads garbage; base-64 wedges the device), hence
    ones-first v_aug layout.
  - fp8/DoubleRow was evaluated and REJECTED: any e4m3 quantization on
    the value path costs ~3e-2 absmax-rel error alone (quantization
    noise in random-sign dot products scales with the signal).
  - QKV q/k parts emitted as half-units (8 MMs, ~1.1us) used with v and
    proj units as PE filler inside attention rounds; only v0-3 + two
    qk halves precede the first scores, so the ACT exp stream (the
    ~70us serial resource) starts as early as the DMAs allow.
  - adaptive filler draining: surplus fillers are consumed 2-per-round
    so none are left at a pair boundary where they would queue ahead
    of the next pair's score matmuls and stall exp.
  - y DMAs rotate over sync/gpsimd queues.

Kept from v2: bf16 pipeline, k-interleaved host packing with 192-256KB
DMA instructions spread across queues, score/exp/PV software pipeline
with per-pair PSUM tile rotation, endgame per-chunk normalize + proj(3)
interleave.
"""

import math

import numpy as np

B, L, D, H = 2, 2048, 1024, 16
KHD = D // H  # 64 head dim
NCORES = 8
HPC = 4  # heads per core
FQK = 2 * HPC * KHD  # 512 q+k local features
FV = HPC * KHD  # 256 v local features
FQKV = FQK + FV  # 768
DK = D // 128  # 8 contraction chunks
LC = L // 128  # 16 row chunks
NJ = L // 512  # 4 qrow blocks
NXB = 4  # xT column blocks of 512
SCALE = 1.0 / math.sqrt(KHD)

_CACHE = {}

# Emission schedule: which filler units run inside which attention pair
# (one unit consumed per round).  v8..v15 are QKV value-chunk units;
# p<j>.<lq><half> are proj halves; None skips a round.
SCHED = {
    "plan": {
        "00": ["q0.01", "q2.01", "q1.00", "q3.00"],
        "02": ["q1.01", "q3.01", "v4"],
        "10": ["v5", "v6", "v7", "v8", "v9"],
        "12": ["v10", "v11", "q0.10", "q2.10", "q0.11"],
        "20": ["q2.11", "q1.10", "q3.10", "v12", "v13", "p0.00", "p0.01"],
        "22": ["q1.11", "q3.11", "v14", "v15", "p0.10", "p0.11", "p0.20"],
        "30": ["p0.21", "p0.30", "p0.31", "p1.00", "p1.01", "p1.10", "p1.11",
               "p1.20", "p1.21"],
        "32": ["p1.30", "p1.31", "p2.00", "p2.01", "p2.10", "p2.11", "p2.20",
               "p2.21", "p2.30", "p2.31"],
    },
    "tail": [],  # proj(3) is emitted inside the endgame pair
}


def _build(has_bqk: bool, has_bv: bool, has_bp: bool, reps: int = 1):
    import concourse.bass as bass
    import concourse.mybir as mybir
    import concourse.tile as tile
    from concourse import bacc

    f32 = mybir.dt.float32
    bf16 = mybir.dt.bfloat16

    nc = bacc.Bacc(None, target_bir_lowering=False)
    # host-packed k-interleaved layouts: one wide DMA per tensor
    xTi_d = nc.declare_dram_parameter("xTi", [128, NXB, DK, 512], bf16, isOutput=False)
    wqkvTi_d = nc.declare_dram_parameter("wqkvTi", [128, DK, FQKV], bf16, isOutput=False)
    wpTi_d = nc.declare_dram_parameter("wpTi", [128, 2, D], bf16, isOutput=False)
    tri_d = nc.declare_dram_parameter("tri", [128, 128], bf16, isOutput=False)
    if has_bqk:
        bqk_d = nc.declare_dram_parameter("bqk", [128, FQK // 128], f32, isOutput=False)
    if has_bv or has_bp:
        onesrow_d = nc.declare_dram_parameter("onesrow", [1, 128], bf16, isOutput=False)
    if has_bv:
        bv_d = nc.declare_dram_parameter("bv", [1, FV], bf16, isOutput=False)
    if has_bp:
        bp_d = nc.declare_dram_parameter("bp", [1, D], bf16, isOutput=False)
    y_d = nc.declare_dram_parameter("y", [L, D], bf16, isOutput=True)

    with nc.allow_low_precision(reason="bf16 matmul pipeline, fp32 accum"), tile.TileContext(nc) as tc:
        with (
            tc.tile_pool(name="persist", bufs=1) as persist,
            tc.tile_pool(name="work", bufs=3) as work,
            tc.tile_pool(name="small", bufs=2) as small,
            tc.tile_pool(name="ps_sc", bufs=2, space="PSUM") as ps_sc,
            tc.tile_pool(name="ps_out", bufs=2, space="PSUM") as ps_out,
            tc.tile_pool(name="ps_y", bufs=2, space="PSUM") as ps_y,
        ):
            for _rep in range(reps):
                # ---- persistent SBUF tensors ----
                xTi_sbs = [
                    persist.tile([128, DK, 512], bf16, name=f"xTi{c}", tag=f"xTi{c}")
                    for c in range(NXB)
                ]
                wqkvTi_sb = persist.tile([128, DK, FQKV], bf16, tag="wqkvTi")
                qT_p = [persist.tile([128, L], bf16, name=f"qT{p}", tag=f"qT{p}") for p in range(2)]
                kT_p = [persist.tile([128, L], bf16, name=f"kT{p}", tag=f"kT{p}") for p in range(2)]
                # v augmented with 64 ones-columns (cols 0:64; v in cols
                # 64:128): the PV matmul then materializes the softmax
                # denominator replicated on outT partitions 0:64, so
                # normalization needs no partition broadcast (gpsimd
                # broadcast measured ~4.8us on HW - 10x the modeled cost -
                # and serialized every pair).  The denominator lives at
                # base partition 0 because custom-DVE ops (the fast
                # reciprocal) are broken/wedge the device at base 64.
                # M=128 streams at the same rate as M=65 (measured 134ns
                # for M=128 N=512 bf16).
                v_augs = [
                    persist.tile([128, HPC, 128], bf16, name=f"vaug{lc}", tag=f"vaug{lc}")
                    for lc in range(LC)
                ]
                attnT_js = [
                    persist.tile([128, 2, 512], bf16, name=f"attnT{j}", tag=f"attnT{j}")
                    for j in range(NJ)
                ]
                wpTi_sb = persist.tile([128, 2, D], bf16, tag="wpTi")
                tri_sb = persist.tile([128, 128], bf16)

                # ---- input DMAs: few, wide, priority-ordered; none on ACT
                # (it needs every cycle for exp).  First emit_v(0..3) needs
                # only xTi[0] cols 0:128-per-k and wqkvTi v-cols: load those
                # first so PE starts ~3us in.
                # ~192-256KB per DMA instruction: each instruction is
                # served by ~one DMA engine (~22-27 GB/s), so aggregate
                # bandwidth comes from many concurrent instructions spread
                # over the sync/scalar HWDGE + gpsimd SWDGE queues.  All
                # slices keep >=1.5KB contiguous rows (128-256 descriptors).
                engs = [nc.sync, nc.scalar, nc.gpsimd]
                # tiny k=0 pieces first: the very first emit_v matmul needs
                # only wqkv v-cols of k=0 and xTi[0] k=0, so PE starts ~2us
                # earlier than waiting for full 192-256KB chunks
                nc.sync.dma_start(
                    out=wqkvTi_sb[:, 0, FQK:FQKV], in_=wqkvTi_d[:, 0, FQK:FQKV]
                )
                nc.scalar.dma_start(
                    out=xTi_sbs[0][:, 0], in_=xTi_d[:, 0, 0]
                )
                nc.gpsimd.dma_start(
                    out=wqkvTi_sb[:, 0, 0:FQK], in_=wqkvTi_d[:, 0, 0:FQK]
                )
                for k in range(1, DK):
                    engs[k % 3].dma_start(
                        out=wqkvTi_sb[:, k], in_=wqkvTi_d[:, k]
                    )
                nc.sync.dma_start(
                    out=xTi_sbs[0][:, 1], in_=xTi_d[:, 0, 1]
                )
                for c in range(NXB):
                    for kp in range(1 if c == 0 else 0, 4):
                        engs[(c + kp) % 3].dma_start(
                            out=xTi_sbs[c][:, 2 * kp:2 * kp + 2],
                            in_=xTi_d[:, c, 2 * kp:2 * kp + 2],
                        )
                for kc in range(2):
                    engs[kc].dma_start(
                        out=wpTi_sb[:, kc], in_=wpTi_d[:, kc]
                    )
                nc.sync.dma_start(out=tri_sb, in_=tri_d[:])
                # ones columns of v_aug (softmax denominator trick,
                # replicated so the denominator lands on 64 partitions)
                for lc in range(LC):
                    nc.vector.memset(v_augs[lc][:, :, 0:KHD], 1.0)
                # PE p-state warm-up: ~2.5us of throwaway matmuls on memset
                # data so the 3us continuous-busy ramp to 2.4GHz completes
                # before real work arrives (first real matmul waits ~3us of
                # DMA anyway; the ramp would otherwise restart on it)
                if _rep == 0:
                    warm_sb = work.tile([128, 128], bf16, tag="warm", bufs=1)
                    nc.vector.memset(warm_sb, 0.0)
                    warm_ps = ps_sc.tile([128, 128], f32, tag="sc", name="warm")
                    for _ in range(27):
                        nc.tensor.matmul(
                            warm_ps, warm_sb, warm_sb, start=True, stop=True
                        )
                if has_bqk:
                    bqk_sb = persist.tile([128, FQK // 128], f32)
                    nc.sync.dma_start(out=bqk_sb, in_=bqk_d[:])
                if has_bv or has_bp:
                    ones_row = persist.tile([1, 128], bf16)
                    nc.sync.dma_start(out=ones_row, in_=onesrow_d[:])
                if has_bv:
                    bv_sb = persist.tile([1, FV], bf16)
                    nc.sync.dma_start(out=bv_sb, in_=bv_d[:])
                if has_bp:
                    bp_sb = persist.tile([1, D], bf16)
                    nc.sync.dma_start(out=bp_sb, in_=bp_d[:])

                # ---- emission helpers ----
                def emit_qk_half(m, n2, half):
                    # q,k feature-major: chunk m covers feats 128m..128m+127;
                    # (n2, half) covers L-columns 1024*n2+512*half..+512.
                    # Half-sized units (~1.1us PE) so a filler never delays
                    # the next round's score matmuls (which gate the ACT
                    # exp stream) by more than ~1us.
                    dst = qT_p if m < 2 else kT_p
                    plane = m % 2
                    ps = ps_sc.tile([128, 1024], f32, tag="sc", name="ps")
                    n = 2 * n2 + half
                    for k in range(DK):
                        nc.tensor.matmul(
                            ps[:, 0:512],
                            wqkvTi_sb[:, k, m * 128:(m + 1) * 128],
                            xTi_sbs[n][:, k, :],
                            start=(k == 0),
                            stop=(k == DK - 1),
                        )
                    csl = slice(n2 * 1024 + half * 512, n2 * 1024 + half * 512 + 512)
                    if has_bqk:
                        nc.scalar.activation(
                            dst[plane][:, csl],
                            ps[:, 0:512],
                            mybir.ActivationFunctionType.Copy,
                            bias=bqk_sb[:, m:m + 1],
                        )
                    else:
                        nc.vector.tensor_copy(
                            out=dst[plane][:, csl], in_=ps[:, 0:512]
                        )

                def emit_v(lc):
                    # v natural layout [L, feat]; uses the proj pool's PSUM
                    # slots (idle during QKV and early attention) so v units
                    # never perturb the score/exp tile rotation
                    psv = ps_y.tile([128, 512], f32, tag="psy", name="psv")
                    c, r = divmod(lc, NXB)
                    for k in range(DK):
                        nc.tensor.matmul(
                            psv[:, 0:FV],
                            xTi_sbs[c][:, k, r * 128:(r + 1) * 128],
                            wqkvTi_sb[:, k, FQK:FQKV],
                            start=(k == 0),
                            stop=(k == DK - 1) and not has_bv,
                        )
                    if has_bv:
                        nc.tensor.matmul(
                            psv[:, 0:FV], ones_row[0:1, :], bv_sb,
                            start=False, stop=True,
                        )
                    nc.vector.tensor_copy(
                        out=v_augs[lc][:, :, KHD:128],
                        in_=psv[:, 0:FV].rearrange("p (h k) -> p h k", h=HPC),
                    )

                def emit_att_pair(j, h0, fillers=(), endgame=False):
                    # two heads (same q/k plane) processed in lockstep, with the
                    # PV matmuls software-pipelined one round behind the scores
                    # so PE never waits on ScalarE's exp.  One filler thunk
                    # (a proj half) is emitted after each round as PE slack.
                    # endgame (last pair): normalize per 128-col chunk as soon
                    # as that chunk's PV accumulation is complete, and emit
                    # proj(NJ-1) column-chunks right behind - shortens the
                    # critical tail chain.
                    fillers = list(fillers)
                    pl = h0 // 2
                    pos = [(h0 % 2) * 64, ((h0 + 1) % 2) * 64]
                    heads = [h0, h0 + 1]
                    outTs = [
                        ps_out.tile([128, 512], f32, tag="outT", name="outT")
                        for _ in range(2)
                    ]
                    qrs = slice(j * 512, (j + 1) * 512)
                    last_c = 4 * j + 3
                    # rounds: below-diagonal pairs then two packed diagonal pairs
                    rounds = [("below", cp) for cp in range(0, 4 * j, 2)]
                    rounds += [("diag", 0), ("diag", 2)]
                    pending = []  # (hh, [(c, colslice in ex)]), ex tile

                    def flush_pending():
                        # endgame: close the accumulation group at c=4j+1 so
                        # cols 0:256 are readable before diag-2 PV lands
                        # (stop is sim bookkeeping only; hardware PSUM
                        # accumulation is per-address)
                        for hh, parts, ex in pending:
                            for c, exsl, n0 in parts:
                                nc.tensor.matmul(
                                    outTs[hh][:, n0:512],
                                    v_augs[c][:, heads[hh], :],
                                    ex[:, exsl],
                                    start=(c == 0),
                                    stop=(c == last_c or (endgame and c == last_c - 2)),
                                    skip_group_check=(endgame and c > last_c - 2),
                                )
                        pending.clear()

                    def emit_norm(s, w):
                        # normalize cols s*w..(s+1)*w of both heads; the
                        # denominator is already replicated on outT rows
                        # 64:128, so this is recip + mul on DVE only
                        csl = slice(s * w, (s + 1) * w)
                        for hh in range(2):
                            po = pos[hh]
                            recip = small.tile([64, w], f32, tag="recip", name="recip")
                            # approx_fast: 18 correct bits (plenty for bf16
                            # attnT), single DVE op, ~5x faster than the
                            # iterative-divide reciprocal (8 cyc/elem); all
                            # APs at base partition 0 (custom-DVE ops give
                            # garbage / wedge the device at base 64)
                            nc.vector.reciprocal_approx_fast(
                                recip, outTs[hh][0:KHD, csl]
                            )
                            nc.vector.tensor_mul(
                                attnT_js[j][po:po + 64, pl, csl],
                                outTs[hh][KHD:128, csl],
                                recip,
                            )

                    for ridx, (kind, arg) in enumerate(rounds):
                        new_pending = []
                        for hh in range(2):
                            po = pos[hh]
                            sc = ps_sc.tile([128, 1024], f32, tag="sc", name="sc")
                            if kind == "below":
                                cp = arg
                                for half in range(2):
                                    c = cp + half
                                    nc.tensor.matmul(
                                        sc[:, half * 512:(half + 1) * 512],
                                        kT_p[pl][po:po + 64, c * 128:(c + 1) * 128],
                                        qT_p[pl][po:po + 64, qrs],
                                        start=True,
                                        stop=True,
                                    )
                                ex = work.tile([128, 1024], bf16, tag="expT", name="ex", bufs=7)
                                nc.scalar.activation(
                                    ex, sc,
                                    mybir.ActivationFunctionType.Exp, scale=SCALE,
                                )
                                parts = [
                                    (cp, slice(0, 512), 0),
                                    (cp + 1, slice(512, 1024), 0),
                                ]
                            else:
                                i0 = arg
                                ws = [512 - 128 * (i0 + di) for di in range(2)]
                                offs = [0, ws[0]]
                                wtot = ws[0] + ws[1]
                                for di in range(2):
                                    c = 4 * j + i0 + di
                                    n0 = 128 * (i0 + di)
                                    nc.tensor.matmul(
                                        sc[:, offs[di]:offs[di] + ws[di]],
                                        kT_p[pl][po:po + 64, c * 128:(c + 1) * 128],
                                        qT_p[pl][po:po + 64, j * 512 + n0:(j + 1) * 512],
                                        start=True,
                                        stop=True,
                                    )
                                ex = work.tile([128, 1024], bf16, tag="expT", name="ex", bufs=7)
                                nc.scalar.activation(
                                    ex[:, 0:wtot], sc[:, 0:wtot],
                                    mybir.ActivationFunctionType.Exp, scale=SCALE,
                                )
                                for di in range(2):
                                    nc.vector.tensor_mul(
                                        ex[:, offs[di]:offs[di] + 128],
                                        ex[:, offs[di]:offs[di] + 128],
                                        tri_sb,
                                    )
                                parts = [
                                    (4 * j + i0, slice(0, ws[0]), 128 * i0),
                                    (
                                        4 * j + i0 + 1,
                                        slice(offs[1], offs[1] + ws[1]),
                                        128 * (i0 + 1),
                                    ),
                                ]
                            new_pending.append((hh, parts, ex))
                        flush_pending()
                        pending.extend(new_pending)
                        # first round has no PV yet (pipelined one behind):
                        # absorb the exp wait with an extra filler; also
                        # drain surplus fillers early so none are left at
                        # the pair boundary (a leftover would sit in the PE
                        # queue ahead of the next pair's score matmuls and
                        # stall the ACT exp stream)
                        remaining = len(rounds) - ridx - 1
                        take = 2 if (ridx == 0 or len(fillers) > remaining) else 1
                        for _ in range(take):
                            if fillers:
                                fu = fillers.pop(0)
                                if fu is not None:
                                    fu()
                        if endgame and ridx == len(rounds) - 1:
                            # diag-0 PV just flushed: cols 0:256 are final
                            for s in range(2):
                                emit_norm(s, 128)
                            for lq in range(2):
                                for h in range(2):
                                    emit_proj_half(NJ - 1, lq, h)
                    flush_pending()
                    for f in fillers:
                        if f is not None:
                            f()
                    # normalize: attnT[f, qrow] = outT[f, qrow] / outT[64, qrow]
                    if endgame:
                        for s in range(2, 4):
                            emit_norm(s, 128)
                        for lq in range(2, 4):
                            for h in range(2):
                                emit_proj_half(NJ - 1, lq, h)
                    elif j == NJ - 1:
                        for s in range(4):
                            emit_norm(s, 128)
                    else:
                        emit_norm(0, 512)

                proj_ysb = {}

                def emit_proj_half(j, lq, half):
                    # one 512-col half of one 128-row chunk of proj(j);
                    # small unit usable as PE filler between att rounds
                    lc = 4 * j + lq
                    if half == 0:
                        proj_ysb[j] = work.tile(
                            [128, 1024], bf16, tag="ysb", name="ysb", bufs=3
                        )
                    ysb = proj_ysb[j]
                    sl = slice(half * 512, (half + 1) * 512)
                    psy = ps_y.tile([128, 512], f32, tag="psy", name="psy")
                    for kc in range(2):
                        nc.tensor.matmul(
                            psy,
                            attnT_js[j][:, kc, lq * 128:(lq + 1) * 128],
                            wpTi_sb[:, kc, sl],
                            start=(kc == 0),
                            stop=(kc == 1) and not has_bp,
                        )
                    if has_bp:
                        nc.tensor.matmul(
                            psy, ones_row[0:1, :], bp_sb[0:1, sl],
                            start=False, stop=True,
                        )
                    if j == NJ - 1 and (lq * 2 + half) % 2 == 1:
                        # tail: ACT is free once exp is done; alternate
                        # copies DVE/ACT so proj(3) is PE-paced
                        nc.scalar.activation(
                            ysb[:, sl], psy, mybir.ActivationFunctionType.Copy
                        )
                    else:
                        nc.vector.tensor_copy(out=ysb[:, sl], in_=psy)
                    # only SP/ACT/gpsimd queues can start DMAs; keep ACT
                    # free for exp - rotate sync/gpsimd
                    yengs = [nc.sync, nc.gpsimd, nc.sync, nc.gpsimd]
                    if j == NJ - 1 and lq == 3:
                        # drain the final block's halves eagerly
                        yengs[half].dma_start(
                            out=y_d[lc * 128:(lc + 1) * 128, sl], in_=ysb[:, sl]
                        )
                    elif half == 1:
                        yengs[lc % 4].dma_start(
                            out=y_d[lc * 128:(lc + 1) * 128, :], in_=ysb
                        )

                def proj_fillers(j):
                    return [
                        (lambda j=j, lq=lq, h=h: emit_proj_half(j, lq, h))
                        for lq in range(4)
                        for h in range(2)
                    ]

                # ---- emission order: emit_v first so PE ramps while xT
                # streams in; attention's exp stream starts early so ACT
                # overlaps the QKV phase; proj halves injected as PE filler
                # between att rounds so the tail is only proj(3) ----
                def vu(lc):
                    return lambda: emit_v(lc)

                def qku(m, n2, half):
                    return lambda: emit_qk_half(m, n2, half)

                # SCHED["plan"]: per att pair, list of filler unit names;
                # remaining proj units run after their position's pair.
                units = {}
                for lc in range(4, LC):
                    units[f"v{lc}"] = vu(lc)
                for m in range(4):
                    for n2 in range(2):
                        for h in range(2):
                            units[f"q{m}.{n2}{h}"] = qku(m, n2, h)
                for pj in range(NJ):
                    for lq in range(4):
                        for h in range(2):
                            units[f"p{pj}.{lq}{h}"] = (
                                lambda j=pj, lq=lq, h=h: emit_proj_half(j, lq, h)
                            )
                plan = SCHED["plan"]

                def F(pair):
                    return [units[nm] if nm else None for nm in plan.get(pair, [])]

                for lc in range(4):
                    emit_v(lc)
                emit_qk_half(0, 0, 0)
                emit_qk_half(2, 0, 0)
                emit_att_pair(0, 0, fillers=F("00"))
                emit_att_pair(0, 2, fillers=F("02"))
                emit_att_pair(1, 0, fillers=F("10"))
                emit_att_pair(1, 2, fillers=F("12"))
                emit_att_pair(2, 0, fillers=F("20"))
                emit_att_pair(2, 2, fillers=F("22"))
                emit_att_pair(3, 0, fillers=F("30"))
                emit_att_pair(3, 2, fillers=F("32"), endgame=True)
                for nm in SCHED["tail"]:
                    units[nm]()

    nc.compile()
    return nc


def kernel(input_BLD, W_attn, b_attn, W_proj, b_proj):
    from ml_dtypes import bfloat16

    input_BLD = np.asarray(input_BLD, dtype=np.float32)
    W_attn = np.asarray(W_attn, dtype=np.float32)
    b_attn = np.asarray(b_attn, dtype=np.float32)
    W_proj = np.asarray(W_proj, dtype=np.float32)
    b_proj = np.asarray(b_proj, dtype=np.float32)

    has_bqk = bool(np.any(b_attn[: 2 * D]))
    has_bv = bool(np.any(b_attn[2 * D:]))
    has_bp = bool(np.any(b_proj))

    key = (has_bqk, has_bv, has_bp)
    if key not in _CACHE:
        _CACHE[key] = _build(*key)
    nc = _CACHE[key]

    tri = (np.arange(128)[None, :] >= np.arange(128)[:, None]).astype(bfloat16)
    in_maps = []
    for c in range(NCORES):
        b, t = divmod(c, 4)
        hs = t * HPC * KHD  # feature offset of this core's heads
        w_loc = np.concatenate(
            [
                W_attn[hs:hs + FV],  # q rows
                W_attn[D + hs:D + hs + FV],  # k rows
                W_attn[2 * D + hs:2 * D + hs + FV],  # v rows
            ],
            axis=0,
        )  # [768, 1024]
        # k-interleaved host packing: xTi[p, c, k, l] = x[b][c*512+l, k*128+p]
        xT = np.ascontiguousarray(input_BLD[b].T.astype(bfloat16))  # [D, L]
        xTi = np.ascontiguousarray(
            xT.reshape(DK, 128, NXB, 512).transpose(1, 2, 0, 3)
        )  # [128, 4, 8, 512]
        wqkvT = w_loc.T.astype(bfloat16)  # [D, 768]
        wqkvTi = np.ascontiguousarray(
            wqkvT.reshape(DK, 128, FQKV).transpose(1, 0, 2)
        )  # [128, 8, 768]
        wpT = W_proj[:, hs:hs + FV].T.astype(bfloat16)  # [256, 1024]
        wpTi = np.ascontiguousarray(
            wpT.reshape(2, 128, D).transpose(1, 0, 2)
        )  # [128, 2, 1024]
        m = {
            "xTi": xTi,
            "wqkvTi": wqkvTi,
            "wpTi": wpTi,
            "tri": tri,
        }
        if has_bqk:
            bqk = np.concatenate([b_attn[hs:hs + FV], b_attn[D + hs:D + hs + FV]])
            m["bqk"] = np.ascontiguousarray(bqk.reshape(FQK // 128, 128).T)
        if has_bv or has_bp:
            m["onesrow"] = np.ones((1, 128), bfloat16)
        if has_bv:
            m["bv"] = b_attn[2 * D + hs:2 * D + hs + FV][None, :].astype(bfloat16)
        if has_bp:
            m["bp"] = (b_proj / 4.0)[None, :].astype(bfloat16)
        in_maps.append(m)

    from concourse.bass_utils import run_bass_kernel_spmd

    globals()["_last_in_maps"] = in_maps
    res = run_bass_kernel_spmd(nc, in_maps, list(range(NCORES)))
    globals()["_LAST_RESULTS"] = res
    out = np.empty((B, L, D), dtype=np.float32)
    for b in range(B):
        acc = res.results[4 * b]["y"].astype(np.float32)
        for t in range(1, 4):
            acc = acc + res.results[4 * b + t]["y"].astype(np.float32)
        out[b] = acc
    return out



# revision 23
# speedup vs baseline: 1.1223x; 1.0714x over previous
"""Causal self-attention Trainium2 Bass kernel (v3, bf16 pipeline).

Problem: nn_CausalSelfAttention (B=2, L=2048, D=1024, H=16 heads, Khd=64).

Sharding (8 cores): data-parallel over B (2 way) x tensor-parallel over
heads (4 way, 4 heads/core).  Each core computes
  qkv_local = x_b @ W_attn_local.T          (c_attn column-sharded)
  attn_local = causal_attention(q,k,v)      (4 heads)
  y_partial  = attn_local @ W_proj_local.T  (c_proj row-sharded)
and the host sums the 4 partials per batch (the row-parallel unshard).

v3 changes over v2 (~190us -> ~130us HW):
  - softmax normalization without gpsimd partition_broadcast: v_aug
    carries 64 REPLICATED ones-columns (cols 0:64; v in 64:128), so the
    PV matmul materializes the denominator on outT partitions 0:64 and
    normalize is reciprocal_approx_fast + tensor_mul on DVE only.
    HW-measured: gpsimd broadcast [64,512] ~4.8us (10x the model) and
    the accurate DVE reciprocal is an iterative 8-cyc/elem op - the old
    chain was the kernel's hidden serializer.  M=128 PV streams at the
    same rate as M=65 (bf16 matmul: 2 cols/cycle, 134ns at N=512).
  - custom-DVE ops (the fast reciprocal) only work at base partition 0
    on HW (cross-base reads garbage; base-64 wedges the device), hence
    ones-first v_aug layout.
  - fp8/DoubleRow was evaluated and REJECTED: any e4m3 quantization on
    the value path costs ~3e-2 absmax-rel error alone (quantization
    noise in random-sign dot products scales with the signal).
  - QKV q/k parts emitted as half-units (8 MMs, ~1.1us) used with v and
    proj units as PE filler inside attention rounds; only v0-3 + two
    qk halves precede the first scores, so the ACT exp stream (the
    ~70us serial resource) starts as early as the DMAs allow.
  - adaptive filler draining: surplus fillers are consumed 2-per-round
    so none are left at a pair boundary where they would queue ahead
    of the next pair's score matmuls and stall exp.
  - y DMAs rotate over sync/gpsimd queues.

Kept from v2: bf16 pipeline, k-interleaved host packing with 192-256KB
DMA instructions spread across queues, score/exp/PV software pipeline
with per-pair PSUM tile rotation, endgame per-chunk normalize + proj(3)
interleave.
"""

import math

import numpy as np

B, L, D, H = 2, 2048, 1024, 16
KHD = D // H  # 64 head dim
NCORES = 8
HPC = 4  # heads per core
FQK = 2 * HPC * KHD  # 512 q+k local features
FV = HPC * KHD  # 256 v local features
FQKV = FQK + FV  # 768
DK = D // 128  # 8 contraction chunks
LC = L // 128  # 16 row chunks
NJ = L // 512  # 4 qrow blocks
NXB = 4  # xT column blocks of 512
SCALE = 1.0 / math.sqrt(KHD)

_CACHE = {}

# Emission schedule: which filler units run inside which attention pair
# (one unit consumed per round).  v8..v15 are QKV value-chunk units;
# p<j>.<lq><half> are proj halves; None skips a round.
SCHED = {
    "plan": {
        "00": ["q0.01", "q2.01", "q1.00", "q3.00"],
        "02": ["q1.01", "q3.01", "v4"],
        "10": ["v5", "v6", "v7", "v8", "v9"],
        "12": ["v10", "v11", "q0.10", "q2.10", "q0.11"],
        "20": ["q2.11", "q1.10", "q3.10", "v12", "v13", "p0.00", "p0.01"],
        "22": ["q1.11", "q3.11", "v14", "v15", "p0.10", "p0.11", "p0.20"],
        "30": ["p0.21", "p0.30", "p0.31", "p1.00", "p1.01", "p1.10", "p1.11",
               "p1.20", "p1.21"],
        "32": ["p1.30", "p1.31", "p2.00", "p2.01", "p2.10", "p2.11", "p2.20",
               "p2.21", "p2.30", "p2.31"],
    },
    "tail": [],  # proj(3) is emitted inside the endgame pair
}


def _build(has_bqk: bool, has_bv: bool, has_bp: bool, reps: int = 1):
    import concourse.bass as bass
    import concourse.mybir as mybir
    import concourse.tile as tile
    from concourse import bacc

    f32 = mybir.dt.float32
    bf16 = mybir.dt.bfloat16

    nc = bacc.Bacc(None, target_bir_lowering=False)
    # host-packed k-interleaved layouts: one wide DMA per tensor
    xTi_d = nc.declare_dram_parameter("xTi", [128, NXB, DK, 512], bf16, isOutput=False)
    wqkvTi_d = nc.declare_dram_parameter("wqkvTi", [128, DK, FQKV], bf16, isOutput=False)
    wpTi_d = nc.declare_dram_parameter("wpTi", [128, 2, D], bf16, isOutput=False)
    tri_d = nc.declare_dram_parameter("tri", [128, 128], bf16, isOutput=False)
    if has_bqk:
        bqk_d = nc.declare_dram_parameter("bqk", [128, FQK // 128], f32, isOutput=False)
    if has_bv or has_bp:
        onesrow_d = nc.declare_dram_parameter("onesrow", [1, 128], bf16, isOutput=False)
    if has_bv:
        bv_d = nc.declare_dram_parameter("bv", [1, FV], bf16, isOutput=False)
    if has_bp:
        bp_d = nc.declare_dram_parameter("bp", [1, D], bf16, isOutput=False)
    y_d = nc.declare_dram_parameter("y", [L, D], bf16, isOutput=True)

    with nc.allow_low_precision(reason="bf16 matmul pipeline, fp32 accum"), tile.TileContext(nc) as tc:
        with (
            tc.tile_pool(name="persist", bufs=1) as persist,
            tc.tile_pool(name="work", bufs=3) as work,
            tc.tile_pool(name="small", bufs=2) as small,
            tc.tile_pool(name="ps_sc", bufs=2, space="PSUM") as ps_sc,
            tc.tile_pool(name="ps_out", bufs=2, space="PSUM") as ps_out,
            tc.tile_pool(name="ps_y", bufs=2, space="PSUM") as ps_y,
        ):
            for _rep in range(reps):
                # ---- persistent SBUF tensors ----
                xTi_sbs = [
                    persist.tile([128, DK, 512], bf16, name=f"xTi{c}", tag=f"xTi{c}")
                    for c in range(NXB)
                ]
                wqkvTi_sb = persist.tile([128, DK, FQKV], bf16, tag="wqkvTi")
                qT_p = [persist.tile([128, L], bf16, name=f"qT{p}", tag=f"qT{p}") for p in range(2)]
                kT_p = [persist.tile([128, L], bf16, name=f"kT{p}", tag=f"kT{p}") for p in range(2)]
                # v augmented with 64 ones-columns (cols 0:64; v in cols
                # 64:128): the PV matmul then materializes the softmax
                # denominator replicated on outT partitions 0:64, so
                # normalization needs no partition broadcast (gpsimd
                # broadcast measured ~4.8us on HW - 10x the modeled cost -
                # and serialized every pair).  The denominator lives at
                # base partition 0 because custom-DVE ops (the fast
                # reciprocal) are broken/wedge the device at base 64.
                # M=128 streams at the same rate as M=65 (measured 134ns
                # for M=128 N=512 bf16).
                v_augs = [
                    persist.tile([128, HPC, 128], bf16, name=f"vaug{lc}", tag=f"vaug{lc}")
                    for lc in range(LC)
                ]
                attnT_js = [
                    persist.tile([128, 2, 512], bf16, name=f"attnT{j}", tag=f"attnT{j}")
                    for j in range(NJ)
                ]
                wpTi_sb = persist.tile([128, 2, D], bf16, tag="wpTi")
                tri_sb = persist.tile([128, 128], bf16)

                # ---- input DMAs: few, wide, priority-ordered; none on ACT
                # (it needs every cycle for exp).  First emit_v(0..3) needs
                # only xTi[0] cols 0:128-per-k and wqkvTi v-cols: load those
                # first so PE starts ~3us in.
                # ~192-256KB per DMA instruction: each instruction is
                # served by ~one DMA engine (~22-27 GB/s), so aggregate
                # bandwidth comes from many concurrent instructions spread
                # over the sync/scalar HWDGE + gpsimd SWDGE queues.  All
                # slices keep >=1.5KB contiguous rows (128-256 descriptors).
                engs = [nc.sync, nc.scalar, nc.gpsimd]
                # tiny k=0 pieces first: the very first emit_v matmul needs
                # only wqkv v-cols of k=0 and xTi[0] k=0, so PE starts ~2us
                # earlier than waiting for full 192-256KB chunks
                nc.sync.dma_start(
                    out=wqkvTi_sb[:, 0, FQK:FQKV], in_=wqkvTi_d[:, 0, FQK:FQKV]
                )
                nc.scalar.dma_start(
                    out=xTi_sbs[0][:, 0], in_=xTi_d[:, 0, 0]
                )
                nc.gpsimd.dma_start(
                    out=wqkvTi_sb[:, 0, 0:FQK], in_=wqkvTi_d[:, 0, 0:FQK]
                )
                for k in range(1, DK):
                    engs[k % 3].dma_start(
                        out=wqkvTi_sb[:, k], in_=wqkvTi_d[:, k]
                    )
                nc.sync.dma_start(
                    out=xTi_sbs[0][:, 1], in_=xTi_d[:, 0, 1]
                )
                for c in range(NXB):
                    for kp in range(1 if c == 0 else 0, 4):
                        engs[(c + kp) % 3].dma_start(
                            out=xTi_sbs[c][:, 2 * kp:2 * kp + 2],
                            in_=xTi_d[:, c, 2 * kp:2 * kp + 2],
                        )
                for kc in range(2):
                    engs[kc].dma_start(
                        out=wpTi_sb[:, kc], in_=wpTi_d[:, kc]
                    )
                nc.sync.dma_start(out=tri_sb, in_=tri_d[:])
                # ones columns of v_aug (softmax denominator trick,
                # replicated so the denominator lands on 64 partitions)
                for lc in range(LC):
                    nc.vector.memset(v_augs[lc][:, :, 0:KHD], 1.0)
                # PE p-state warm-up: ~2.5us of throwaway matmuls on memset
                # data so the 3us continuous-busy ramp to 2.4GHz completes
                # before real work arrives (first real matmul waits ~3us of
                # DMA anyway; the ramp would otherwise restart on it)
                if _rep == 0:
                    warm_sb = work.tile([128, 128], bf16, tag="warm", bufs=1)
                    nc.vector.memset(warm_sb, 0.0)
                    warm_ps = ps_sc.tile([128, 128], f32, tag="sc", name="warm")
                    for _ in range(27):
                        nc.tensor.matmul(
                            warm_ps, warm_sb, warm_sb, start=True, stop=True
                        )
                if has_bqk:
                    bqk_sb = persist.tile([128, FQK // 128], f32)
                    nc.sync.dma_start(out=bqk_sb, in_=bqk_d[:])
                if has_bv or has_bp:
                    ones_row = persist.tile([1, 128], bf16)
                    nc.sync.dma_start(out=ones_row, in_=onesrow_d[:])
                if has_bv:
                    bv_sb = persist.tile([1, FV], bf16)
                    nc.sync.dma_start(out=bv_sb, in_=bv_d[:])
                if has_bp:
                    bp_sb = persist.tile([1, D], bf16)
                    nc.sync.dma_start(out=bp_sb, in_=bp_d[:])

                # ---- emission helpers ----
                def emit_qk_half(m, n2, half):
                    # q,k feature-major: chunk m covers feats 128m..128m+127;
                    # (n2, half) covers L-columns 1024*n2+512*half..+512.
                    # Half-sized units (~1.1us PE) so a filler never delays
                    # the next round's score matmuls (which gate the ACT
                    # exp stream) by more than ~1us.
                    dst = qT_p if m < 2 else kT_p
                    plane = m % 2
                    ps = ps_sc.tile([128, 1024], f32, tag="sc", name="ps")
                    n = 2 * n2 + half
                    for k in range(DK):
                        nc.tensor.matmul(
                            ps[:, 0:512],
                            wqkvTi_sb[:, k, m * 128:(m + 1) * 128],
                            xTi_sbs[n][:, k, :],
                            start=(k == 0),
                            stop=(k == DK - 1),
                        )
                    csl = slice(n2 * 1024 + half * 512, n2 * 1024 + half * 512 + 512)
                    if has_bqk:
                        nc.scalar.activation(
                            dst[plane][:, csl],
                            ps[:, 0:512],
                            mybir.ActivationFunctionType.Copy,
                            bias=bqk_sb[:, m:m + 1],
                        )
                    else:
                        nc.vector.tensor_copy(
                            out=dst[plane][:, csl], in_=ps[:, 0:512]
                        )

                def emit_v(lc):
                    # v natural layout [L, feat]; uses the proj pool's PSUM
                    # slots (idle during QKV and early attention) so v units
                    # never perturb the score/exp tile rotation
                    psv = ps_y.tile([128, 512], f32, tag="psy", name="psv")
                    c, r = divmod(lc, NXB)
                    for k in range(DK):
                        nc.tensor.matmul(
                            psv[:, 0:FV],
                            xTi_sbs[c][:, k, r * 128:(r + 1) * 128],
                            wqkvTi_sb[:, k, FQK:FQKV],
                            start=(k == 0),
                            stop=(k == DK - 1) and not has_bv,
                        )
                    if has_bv:
                        nc.tensor.matmul(
                            psv[:, 0:FV], ones_row[0:1, :], bv_sb,
                            start=False, stop=True,
                        )
                    nc.vector.tensor_copy(
                        out=v_augs[lc][:, :, KHD:128],
                        in_=psv[:, 0:FV].rearrange("p (h k) -> p h k", h=HPC),
                    )

                def emit_att_pair(j, h0, fillers=(), endgame=False):
                    # two heads (same q/k plane) processed in lockstep, with the
                    # PV matmuls software-pipelined one round behind the scores
                    # so PE never waits on ScalarE's exp.  One filler thunk
                    # (a proj half) is emitted after each round as PE slack.
                    # endgame (last pair): normalize per 128-col chunk as soon
                    # as that chunk's PV accumulation is complete, and emit
                    # proj(NJ-1) column-chunks right behind - shortens the
                    # critical tail chain.
                    fillers = list(fillers)
                    pl = h0 // 2
                    pos = [(h0 % 2) * 64, ((h0 + 1) % 2) * 64]
                    heads = [h0, h0 + 1]
                    outTs = [
                        ps_out.tile([128, 512], f32, tag="outT", name="outT")
                        for _ in range(2)
                    ]
                    qrs = slice(j * 512, (j + 1) * 512)
                    last_c = 4 * j + 3
                    # rounds: below-diagonal pairs then two packed diagonal pairs
                    rounds = [("below", cp) for cp in range(0, 4 * j, 2)]
                    rounds += [("diag", 0), ("diag", 2)]
                    pending = []  # (hh, [(c, colslice in ex)]), ex tile

                    def flush_pending():
                        # endgame: close the accumulation group at c=4j+1 so
                        # cols 0:256 are readable before diag-2 PV lands
                        # (stop is sim bookkeeping only; hardware PSUM
                        # accumulation is per-address)
                        for hh, parts, ex in pending:
                            for c, exsl, n0 in parts:
                                nc.tensor.matmul(
                                    outTs[hh][:, n0:512],
                                    v_augs[c][:, heads[hh], :],
                                    ex[:, exsl],
                                    start=(c == 0),
                                    stop=(c == last_c or (endgame and c == last_c - 2)),
                                    skip_group_check=(endgame and c > last_c - 2),
                                )
                        pending.clear()

                    def emit_norm(s, w):
                        # normalize cols s*w..(s+1)*w of both heads; the
                        # denominator is already replicated on outT rows
                        # 64:128, so this is recip + mul on DVE only
                        csl = slice(s * w, (s + 1) * w)
                        for hh in range(2):
                            po = pos[hh]
                            recip = small.tile([64, w], f32, tag="recip", name="recip")
                            # approx_fast: 18 correct bits (plenty for bf16
                            # attnT), single DVE op, ~5x faster than the
                            # iterative-divide reciprocal (8 cyc/elem); all
                            # APs at base partition 0 (custom-DVE ops give
                            # garbage / wedge the device at base 64)
                            nc.vector.reciprocal_approx_fast(
                                recip, outTs[hh][0:KHD, csl]
                            )
                            nc.vector.tensor_mul(
                                attnT_js[j][po:po + 64, pl, csl],
                                outTs[hh][KHD:128, csl],
                                recip,
                            )

                    for ridx, (kind, arg) in enumerate(rounds):
                        new_pending = []
                        for hh in range(2):
                            po = pos[hh]
                            sc = ps_sc.tile([128, 1024], f32, tag="sc", name="sc")
                            if kind == "below":
                                cp = arg
                                for half in range(2):
                                    c = cp + half
                                    nc.tensor.matmul(
                                        sc[:, half * 512:(half + 1) * 512],
                                        kT_p[pl][po:po + 64, c * 128:(c + 1) * 128],
                                        qT_p[pl][po:po + 64, qrs],
                                        start=True,
                                        stop=True,
                                    )
                                ex = work.tile([128, 1024], bf16, tag="expT", name="ex", bufs=7)
                                nc.scalar.activation(
                                    ex, sc,
                                    mybir.ActivationFunctionType.Exp, scale=SCALE,
                                )
                                parts = [
                                    (cp, slice(0, 512), 0),
                                    (cp + 1, slice(512, 1024), 0),
                                ]
                            else:
                                i0 = arg
                                ws = [512 - 128 * (i0 + di) for di in range(2)]
                                offs = [0, ws[0]]
                                wtot = ws[0] + ws[1]
                                for di in range(2):
                                    c = 4 * j + i0 + di
                                    n0 = 128 * (i0 + di)
                                    nc.tensor.matmul(
                                        sc[:, offs[di]:offs[di] + ws[di]],
                                        kT_p[pl][po:po + 64, c * 128:(c + 1) * 128],
                                        qT_p[pl][po:po + 64, j * 512 + n0:(j + 1) * 512],
                                        start=True,
                                        stop=True,
                                    )
                                ex = work.tile([128, 1024], bf16, tag="expT", name="ex", bufs=7)
                                nc.scalar.activation(
                                    ex[:, 0:wtot], sc[:, 0:wtot],
                                    mybir.ActivationFunctionType.Exp, scale=SCALE,
                                )
                                for di in range(2):
                                    nc.vector.tensor_mul(
                                        ex[:, offs[di]:offs[di] + 128],
                                        ex[:, offs[di]:offs[di] + 128],
                                        tri_sb,
                                    )
                                parts = [
                                    (4 * j + i0, slice(0, ws[0]), 128 * i0),
                                    (
                                        4 * j + i0 + 1,
                                        slice(offs[1], offs[1] + ws[1]),
                                        128 * (i0 + 1),
                                    ),
                                ]
                            new_pending.append((hh, parts, ex))
                        flush_pending()
                        pending.extend(new_pending)
                        # first round has no PV yet (pipelined one behind):
                        # absorb the exp wait with an extra filler; also
                        # drain surplus fillers early so none are left at
                        # the pair boundary (a leftover would sit in the PE
                        # queue ahead of the next pair's score matmuls and
                        # stall the ACT exp stream)
                        remaining = len(rounds) - ridx - 1
                        take = 2 if (ridx == 0 or len(fillers) > remaining) else 1
                        for _ in range(take):
                            if fillers:
                                fu = fillers.pop(0)
                                if fu is not None:
                                    fu()
                        if endgame and ridx == len(rounds) - 1:
                            # diag-0 PV just flushed: cols 0:256 are final
                            for s in range(2):
                                emit_norm(s, 128)
                            for lq in range(2):
                                for h in range(2):
                                    emit_proj_half(NJ - 1, lq, h)
                    flush_pending()
                    for f in fillers:
                        if f is not None:
                            f()
                    # normalize: attnT[f, qrow] = outT[f, qrow] / outT[64, qrow]
                    if endgame:
                        for s in range(2, 4):
                            emit_norm(s, 128)
                        for lq in range(2, 4):
                            for h in range(2):
                                emit_proj_half(NJ - 1, lq, h)
                    elif j == NJ - 1:
                        for s in range(4):
                            emit_norm(s, 128)
                    else:
                        emit_norm(0, 512)

                proj_ysb = {}

                def emit_proj_half(j, lq, half):
                    # one 512-col half of one 128-row chunk of proj(j);
                    # small unit usable as PE filler between att rounds
                    lc = 4 * j + lq
                    if half == 0:
                        proj_ysb[j] = work.tile(
                            [128, 1024], bf16, tag="ysb", name="ysb", bufs=3
                        )
                    ysb = proj_ysb[j]
                    sl = slice(half * 512, (half + 1) * 512)
                    psy = ps_y.tile([128, 512], f32, tag="psy", name="psy")
                    for kc in range(2):
                        nc.tensor.matmul(
                            psy,
                            attnT_js[j][:, kc, lq * 128:(lq + 1) * 128],
                            wpTi_sb[:, kc, sl],
                            start=(kc == 0),
                            stop=(kc == 1) and not has_bp,
                        )
                    if has_bp:
                        nc.tensor.matmul(
                            psy, ones_row[0:1, :], bp_sb[0:1, sl],
                            start=False, stop=True,
                        )
                    if j == NJ - 1 and (lq * 2 + half) % 2 == 1:
                        # tail: ACT is free once exp is done; alternate
                        # copies DVE/ACT so proj(3) is PE-paced
                        nc.scalar.activation(
                            ysb[:, sl], psy, mybir.ActivationFunctionType.Copy
                        )
                    else:
                        nc.vector.tensor_copy(out=ysb[:, sl], in_=psy)
                    # only SP/ACT/gpsimd queues can start DMAs; keep ACT
                    # free for exp - rotate sync/gpsimd
                    yengs = [nc.sync, nc.gpsimd, nc.sync, nc.gpsimd]
                    if j == NJ - 1:
                        # endgame blocks: the y DMAs are on the critical
                        # tail (a 256KB DMA instruction drains at ~25GB/s
                        # ~= 10us on one engine).  Emit per-half as two
                        # 64KB quarters on both queues so each drains in
                        # ~2.6us of parallel DMA.
                        for qtr in range(2):
                            qsl = slice(
                                half * 512 + qtr * 256,
                                half * 512 + qtr * 256 + 256,
                            )
                            yengs[qtr].dma_start(
                                out=y_d[lc * 128:(lc + 1) * 128, qsl],
                                in_=ysb[:, qsl],
                            )
                    elif half == 1:
                        yengs[lc % 4].dma_start(
                            out=y_d[lc * 128:(lc + 1) * 128, :], in_=ysb
                        )

                def proj_fillers(j):
                    return [
                        (lambda j=j, lq=lq, h=h: emit_proj_half(j, lq, h))
                        for lq in range(4)
                        for h in range(2)
                    ]

                # ---- emission order: emit_v first so PE ramps while xT
                # streams in; attention's exp stream starts early so ACT
                # overlaps the QKV phase; proj halves injected as PE filler
                # between att rounds so the tail is only proj(3) ----
                def vu(lc):
                    return lambda: emit_v(lc)

                def qku(m, n2, half):
                    return lambda: emit_qk_half(m, n2, half)

                # SCHED["plan"]: per att pair, list of filler unit names;
                # remaining proj units run after their position's pair.
                units = {}
                for lc in range(4, LC):
                    units[f"v{lc}"] = vu(lc)
                for m in range(4):
                    for n2 in range(2):
                        for h in range(2):
                            units[f"q{m}.{n2}{h}"] = qku(m, n2, h)
                for pj in range(NJ):
                    for lq in range(4):
                        for h in range(2):
                            units[f"p{pj}.{lq}{h}"] = (
                                lambda j=pj, lq=lq, h=h: emit_proj_half(j, lq, h)
                            )
                plan = SCHED["plan"]

                def F(pair):
                    return [units[nm] if nm else None for nm in plan.get(pair, [])]

                for lc in range(4):
                    emit_v(lc)
                emit_qk_half(0, 0, 0)
                emit_qk_half(2, 0, 0)
                emit_att_pair(0, 0, fillers=F("00"))
                emit_att_pair(0, 2, fillers=F("02"))
                emit_att_pair(1, 0, fillers=F("10"))
                emit_att_pair(1, 2, fillers=F("12"))
                emit_att_pair(2, 0, fillers=F("20"))
                emit_att_pair(2, 2, fillers=F("22"))
                emit_att_pair(3, 0, fillers=F("30"))
                emit_att_pair(3, 2, fillers=F("32"), endgame=True)
                for nm in SCHED["tail"]:
                    units[nm]()

    nc.compile()
    return nc


def kernel(input_BLD, W_attn, b_attn, W_proj, b_proj):
    from ml_dtypes import bfloat16

    input_BLD = np.asarray(input_BLD, dtype=np.float32)
    W_attn = np.asarray(W_attn, dtype=np.float32)
    b_attn = np.asarray(b_attn, dtype=np.float32)
    W_proj = np.asarray(W_proj, dtype=np.float32)
    b_proj = np.asarray(b_proj, dtype=np.float32)

    has_bqk = bool(np.any(b_attn[: 2 * D]))
    has_bv = bool(np.any(b_attn[2 * D:]))
    has_bp = bool(np.any(b_proj))

    key = (has_bqk, has_bv, has_bp)
    if key not in _CACHE:
        _CACHE[key] = _build(*key)
    nc = _CACHE[key]

    tri = (np.arange(128)[None, :] >= np.arange(128)[:, None]).astype(bfloat16)
    in_maps = []
    for c in range(NCORES):
        b, t = divmod(c, 4)
        hs = t * HPC * KHD  # feature offset of this core's heads
        w_loc = np.concatenate(
            [
                W_attn[hs:hs + FV],  # q rows
                W_attn[D + hs:D + hs + FV],  # k rows
                W_attn[2 * D + hs:2 * D + hs + FV],  # v rows
            ],
            axis=0,
        )  # [768, 1024]
        # k-interleaved host packing: xTi[p, c, k, l] = x[b][c*512+l, k*128+p]
        xT = np.ascontiguousarray(input_BLD[b].T.astype(bfloat16))  # [D, L]
        xTi = np.ascontiguousarray(
            xT.reshape(DK, 128, NXB, 512).transpose(1, 2, 0, 3)
        )  # [128, 4, 8, 512]
        wqkvT = w_loc.T.astype(bfloat16)  # [D, 768]
        wqkvTi = np.ascontiguousarray(
            wqkvT.reshape(DK, 128, FQKV).transpose(1, 0, 2)
        )  # [128, 8, 768]
        wpT = W_proj[:, hs:hs + FV].T.astype(bfloat16)  # [256, 1024]
        wpTi = np.ascontiguousarray(
            wpT.reshape(2, 128, D).transpose(1, 0, 2)
        )  # [128, 2, 1024]
        m = {
            "xTi": xTi,
            "wqkvTi": wqkvTi,
            "wpTi": wpTi,
            "tri": tri,
        }
        if has_bqk:
            bqk = np.concatenate([b_attn[hs:hs + FV], b_attn[D + hs:D + hs + FV]])
            m["bqk"] = np.ascontiguousarray(bqk.reshape(FQK // 128, 128).T)
        if has_bv or has_bp:
            m["onesrow"] = np.ones((1, 128), bfloat16)
        if has_bv:
            m["bv"] = b_attn[2 * D + hs:2 * D + hs + FV][None, :].astype(bfloat16)
        if has_bp:
            m["bp"] = (b_proj / 4.0)[None, :].astype(bfloat16)
        in_maps.append(m)

    from concourse.bass_utils import run_bass_kernel_spmd

    globals()["_last_in_maps"] = in_maps
    res = run_bass_kernel_spmd(nc, in_maps, list(range(NCORES)))
    globals()["_LAST_RESULTS"] = res
    out = np.empty((B, L, D), dtype=np.float32)
    for b in range(B):
        acc = res.results[4 * b]["y"].astype(np.float32)
        for t in range(1, 4):
            acc = acc + res.results[4 * b + t]["y"].astype(np.float32)
        out[b] = acc
    return out

